# revision 17
# baseline (speedup 1.0000x reference)
"""Channel-attention module (CAM) kernel for Trainium2.

Reference computation (per batch b):
    a    = x[b].reshape(HW, C)                      # [4096, 512]
    aTa  = a.T @ a                                  # [512, 512]
    attn = softmax(aTa, axis=-1)
    y    = a @ attn                                 # [4096, 512]
    out[b] = gamma * y + x[b]

Sharding: data-parallel over batch B=16 across 8 NeuronCores (2 batches
per core), gamma replicated.  No collectives needed.

Per-core schedule (PE-bound kernel; the whole game is keeping the PE at
2.4 GHz with zero idle):

    warmup  ~10 throwaway f32r matmuls issued as soon as the engine
            barrier clears (dep: one gpsimd memset).  They keep the PE
            busy through the HAM activity window so the clock gate flips
            to 8/8 at ~10us instead of ~21us.
    b0      fused p1+tp chunk loop: each arriving 128-row chunk does its
            4 symmetric-aTa matmuls (bf16) AND its 4 PE transposes
            (f32r) immediately.  The chunk DMA cadence (~650ns) is
            slower than p1 alone (~545ns) but faster than p1+tp
            (~890ns), so fusing converts DMA-wait idle into work and
            front-loads all of b0's transposes into the load phase.
    b0 tail evac aTa + mirrors; softmax(b0) runs on DVE/ACT while the PE
            starts batch-1 pass-1 chunks (b1 data is arriving by then).
    p2(b0)  32 chunks x 4 f32r matmuls vs attn (LDW hides under the
            512-wide streams); PSUM evac on DVE, out-DMA issued from the
            Scalar/Activation HWDGE queue so the Sync queue stays
            dedicated to input loads (no FIFO cross-blocking).
    b1      remaining p1 chunks; then mirrors; transposes of b1 run
            *after* p1 so they cover the softmax(b1) bubble on the PE;
            then p2(b1).

  pass 1  aTa is symmetric: only diagonal+upper blocks are computed
          (rhs free dim 512/384/256/128 per column-block), lower blocks
          mirrored via 6 PE transposes.  Runs in bf16 (fast weight load;
          softmax(aTa) is insensitive to aTa precision because the ~HW
          diagonal towers over off-diagonal entries for this operator).
  softmax folds gamma into the normalizer and adds I so pass 2 directly
          yields gamma*y + a = a @ (g*attn + I)
  tpose   a -> aT via PE transpose (128x128 f32r blocks, 4 per PSUM
          bank), evacuated into one [128, 4, HW] f32r tile.
  pass 2  y[k] (PSUM) += aT[cb-block] @ attn'[cb]; copy to SBUF, DMA out.

Pass-2 / transpose operands are float32r (fp32 truncated by the PE to
~22 bits, streaming 1 column/cycle like bf16), keeping rel err ~2e-4.

Baseline (previous session): 142.2us.  This schedule: see test log.
"""

import numpy as np

import concourse.bacc as bacc
import concourse.mybir as mybir
import concourse.tile as tile
from concourse.bass_utils import run_bass_kernel_spmd
from concourse.masks import make_identity

B, H, W, C = 16, 64, 64, 512
HW = H * W                      # 4096
NCORES = 8
BPC = B // NCORES               # batches per core
NT = HW // 128                  # 32 row-chunks of a
CB = C // 128                   # 4 column-blocks of C
F32 = mybir.dt.float32
F32R = mybir.dt.float32r
BF16 = mybir.dt.bfloat16


def build_bass():
    nc = bacc.Bacc("TRN2", target_bir_lowering=False, debug=False)
    x = nc.dram_tensor("x", [BPC, HW, C], F32, kind="ExternalInput").ap()
    gamma = nc.dram_tensor("gamma", [1], F32, kind="ExternalInput").ap()
    out = nc.dram_tensor("out", [BPC, HW, C], F32, kind="ExternalOutput").ap()

    with tile.TileContext(nc) as tc:
        with (
            tc.tile_pool(name="singles", bufs=1) as singles,
            tc.tile_pool(name="a", bufs=34) as a_pool,
            tc.tile_pool(name="at", bufs=1) as at_pool,
            tc.tile_pool(name="atasb", bufs=6) as atasb_pool,
            tc.tile_pool(name="attn", bufs=8) as attn_pool,
            tc.tile_pool(name="stats", bufs=16) as stats_pool,
            tc.tile_pool(name="ostage", bufs=6) as out_pool,
            tc.tile_pool(name="abf", bufs=21) as bf_pool,
            tc.tile_pool(name="psum", bufs=8, space="PSUM") as psum_pool,
        ):
            # PE warmup: depends only on one gpsimd memset, so it starts
            # right after the engine preamble barrier and keeps the PE
            # busy through a full HAM activity window -> clock flips to
            # 8/8 before the real pass-1 work begins.
            warm_f = singles.tile([128, 512], F32)
            nc.gpsimd.memset(warm_f, 0.0)
            warm = warm_f.bitcast(F32R)
            wps = psum_pool.tile([128, C], F32, tag="ps")
            for _ in range(10):
                nc.tensor.matmul(
                    wps, warm[:, :128], warm, start=True, stop=True
                )

            ident = singles.tile([128, 128], F32)
            make_identity(nc, ident)
            ident_r = singles.tile([128, 128], F32R)
            # on ACT, not DVE: DVE is strict FIFO and this copy waits on
            # gpsimd's make_identity — it would stall the pass-1 casts
            nc.scalar.copy(ident_r, ident)
            gam = singles.tile([128, 1], F32)
            nc.gpsimd.dma_start(out=gam, in_=gamma.to_broadcast((128, 1)))

            st = [dict() for _ in range(BPC)]   # per-batch tile state

            def alloc_at(b):
                # single [128, CB, HW] f32r tile: one strided evac per
                # transpose bank; pool bufs=1 so b1 reuses b0's buffer
                # once p2(b0) has consumed it.
                st[b]["at"] = at_pool.tile(
                    [128, CB, HW], F32R, tag="at", name="at"
                )

            def load_chunk(b, k):
                """Issue the input DMA for chunk k of batch b (Sync)."""
                s = st[b]
                ak = a_pool.tile([128, C], F32R, tag="a", name="a")
                nc.sync.dma_start(
                    out=ak,
                    in_=x[b, k * 128:(k + 1) * 128, :].bitcast(F32R),
                )
                s.setdefault("a", {})[k] = ak

            def cast_chunk(b, k, on_act=False):
                s = st[b]
                ab = bf_pool.tile([128, C], BF16, tag="abf", name="abf")
                if on_act:
                    nc.scalar.copy(ab, s["a"][k].bitcast(F32))
                else:
                    nc.vector.tensor_copy(ab, s["a"][k].bitcast(F32))
                s.setdefault("abf", {})[k] = ab

            def p1_chunk(b, k, fuse_tp=False):
                """4 pass-1 matmuls (bf16) for chunk k; load/cast must
                already be emitted."""
                s = st[b]
                if k == 0:
                    s["ata"] = [
                        psum_pool.tile([128, C], F32, tag="ps", name="ata")
                        for _ in range(CB)
                    ]
                ab = s["abf"][k]
                for cb in range(CB):
                    nc.tensor.matmul(
                        s["ata"][cb][:, cb * 128:C],
                        ab[:, cb * 128:(cb + 1) * 128],
                        ab[:, cb * 128:C],
                        start=(k == 0),
                        stop=(k == NT - 1),
                    )
                if fuse_tp:
                    tp_chunk(b, k)

            def bridge(n):
                """Throwaway warm matmuls that keep the PE busy (and the
                HAM activity monitor ticking) across a phase transition
                while DVE/ACT drain evacuation queues."""
                bps = psum_pool.tile([128, C], F32, tag="ps", name="bridge")
                for _ in range(n):
                    nc.tensor.matmul(
                        bps, warm[:, :128], warm, start=True, stop=True
                    )

            def tp_chunk(b, k):
                """4 PE transposes of chunk k -> one PSUM bank -> one
                strided evac into the [128, CB, HW] aT tile.

                The evac always runs on ACT: in the fused b0 loop the DVE
                already carries the bf16 cast (476ns) and cast+evac would
                exceed the ~890ns chunk cadence and stall the PE."""
                s = st[b]
                tp = psum_pool.tile([128, C], F32R, tag="ps", name="tp")
                for cb in range(CB):
                    nc.tensor.transpose(
                        tp[:, cb * 128:(cb + 1) * 128],
                        s["a"][k][:, cb * 128:(cb + 1) * 128],
                        ident_r,
                    )
                dst = s["at"][:, :, k * 128:(k + 1) * 128]
                src = tp.bitcast(F32).rearrange("p (c w) -> p c w", c=CB)
                nc.scalar.copy(dst, src)

            def evac_msrc(b):
                """Stage mirror sources + evacuate diag+upper aTa,
                row-ordered so each aTa PSUM bank frees as early as
                possible (the next batch's aTa accumulators reuse them)."""
                s = st[b]
                s["msrc"] = {}
                s["asb"] = [
                    atasb_pool.tile([128, C], F32, tag="atasb", name="asb")
                    for _ in range(CB)
                ]
                eng = 0
                for db in range(CB):
                    # all readers of aTa row db, back to back
                    for cb in range(db + 1, CB):
                        m = atasb_pool.tile(
                            [128, 128], F32R, tag="msrc", name="msrc", bufs=8
                        )
                        if eng % 2 == 0:
                            nc.vector.tensor_copy(
                                m, s["ata"][db][:, cb * 128:(cb + 1) * 128]
                            )
                        else:
                            nc.scalar.copy(
                                m, s["ata"][db][:, cb * 128:(cb + 1) * 128]
                            )
                        eng += 1
                        s["msrc"][(cb, db)] = m
                    if db % 2 == 0:
                        nc.vector.tensor_copy(
                            s["asb"][db][:, db * 128:C],
                            s["ata"][db][:, db * 128:C],
                        )
                    else:
                        nc.scalar.copy(
                            s["asb"][db][:, db * 128:C],
                            s["ata"][db][:, db * 128:C],
                        )

            def mirrors(b):
                """Fill lower aTa blocks: (cb, db) = (db, cb)^T via PE."""
                s = st[b]
                for cb in range(CB):
                    for db in range(cb):
                        mir = psum_pool.tile(
                            [128, 128], F32R, tag="ps", name="mir"
                        )
                        nc.tensor.transpose(mir, s["msrc"][(cb, db)], ident_r)
                        if (cb + db) % 2 == 0:
                            nc.vector.tensor_copy(
                                s["asb"][cb][:, db * 128:(db + 1) * 128],
                                mir.bitcast(F32),
                            )
                        else:
                            nc.scalar.copy(
                                s["asb"][cb][:, db * 128:(db + 1) * 128],
                                mir.bitcast(F32),
                            )

            def softmax(b, cbs=range(CB)):
                s = st[b]
                s.setdefault("attn", [])
                for cb in cbs:
                    asb = s["asb"][cb]
                    negmax = stats_pool.tile([128, 1], F32, tag="st")
                    nc.vector.reduce_max(
                        negmax, asb, axis=mybir.AxisListType.X, negate=True
                    )
                    rowsum = stats_pool.tile([128, 1], F32, tag="st")
                    nc.scalar.activation(
                        asb,
                        asb,
                        mybir.ActivationFunctionType.Exp,
                        bias=negmax,
                        scale=1.0,
                        accum_out=rowsum,
                    )
                    grec = stats_pool.tile([128, 1], F32, tag="st")
                    nc.vector.reciprocal(grec, rowsum)
                    # fold gamma into the row normalizer: attn' = g/rowsum * E
                    nc.vector.tensor_scalar_mul(grec, grec, gam)
                    nc.vector.tensor_scalar_mul(asb, asb, grec)
                    # + I on the diagonal block so pass2 fuses the residual
                    nc.vector.tensor_add(
                        asb[:, cb * 128:(cb + 1) * 128],
                        asb[:, cb * 128:(cb + 1) * 128],
                        ident,
                    )
                    ar = attn_pool.tile([128, C], F32R, tag="attn")
                    if cb % 2 == 0:
                        nc.vector.tensor_copy(ar, asb)
                    else:
                        nc.scalar.copy(ar, asb)
                    s["attn"].append(ar)

            def p2_chunk(b, k):
                s = st[b]
                yp = psum_pool.tile([128, C], F32, tag="ps", name="yp")
                for cb in range(CB):
                    nc.tensor.matmul(
                        yp,
                        s["at"][:, cb, k * 128:(k + 1) * 128],
                        s["attn"][cb],
                        start=(cb == 0),
                        stop=(cb == CB - 1),
                    )
                o = out_pool.tile([128, C], F32, tag="o", name="o")
                nc.vector.tensor_copy(o, yp)
                # out-DMA on the ACT HWDGE queue: keeps the Sync queue
                # free for input loads (no FIFO cross-blocking).
                nc.scalar.dma_start(
                    out=out[b, k * 128:(k + 1) * 128, :], in_=o
                )

            # ---------------- braided two-batch schedule ----------------
            # PE order: fused p1+tp(b0) | bridge | mirrors(b0) | p1(b1)
            # (covers softmax(b0)) | p2(b0) start | mirrors(b1) | rest of
            # p2(b0) (covers softmax(b1), emitted interleaved to avoid
            # DVE FIFO head-blocking) | tp(b1) braided 1:4 with p2(b1)
            # (pure-transpose blocks don't tick the HAM activity monitor
            # and get the PE clock re-throttled).
            alloc_at(0)
            for k in range(NT):
                load_chunk(0, k)
                if k >= 8:
                    # b1 loads ride behind b0's on the Sync queue; b0 is
                    # fully buffered in SBUF so the interleave is safe
                    load_chunk(1, k - 8)
                cast_chunk(0, k)
                p1_chunk(0, k, fuse_tp=True)
            # keep the PE busy while DVE/ACT drain the aTa evacuations
            bridge(4)
            evac_msrc(0)
            mirrors(0)
            # pre-cast the first b1 chunks so p1(b1) isn't FIFO-blocked
            # behind softmax(b0) on DVE/ACT
            for j in range(12):
                cast_chunk(1, j, on_act=(j % 2 == 1))
            softmax(0)
            for k in range(NT):
                if k >= 24:
                    load_chunk(1, k)
                if k >= 12:
                    cast_chunk(1, k, on_act=(k % 2 == 1))
                p1_chunk(1, k)
            for k in range(3):
                p2_chunk(0, k)
            evac_msrc(1)
            mirrors(1)
            alloc_at(1)
            for k in range(3, NT):
                p2_chunk(0, k)
                # softmax(b1) interleaved: its DVE/ACT ops trickle in
                # between the p2 evacs instead of head-blocking them
                if k in (6, 9, 12, 15):
                    softmax(1, cbs=[(k - 6) // 3])
                if k == 29:
                    tp_chunk(1, 0)
                if k == 30:
                    tp_chunk(1, 1)
            for j in (2, 3):
                tp_chunk(1, j)
            for k in range(NT):
                if k + 4 < NT:
                    tp_chunk(1, k + 4)
                p2_chunk(1, k)

    nc.compile()
    return nc


_NC_CACHE = None


def _get_nc():
    global _NC_CACHE
    if _NC_CACHE is None:
        _NC_CACHE = build_bass()
    return _NC_CACHE


def make_in_maps(x: np.ndarray, gamma: np.ndarray):
    x = np.ascontiguousarray(np.asarray(x, dtype=np.float32)).reshape(B, HW, C)
    gamma = np.ascontiguousarray(np.asarray(gamma, dtype=np.float32)).reshape(1)
    return [
        {"x": x[i * BPC:(i + 1) * BPC], "gamma": gamma} for i in range(NCORES)
    ]


def kernel(x: np.ndarray, gamma: np.ndarray, _trace: bool = False, _tmpdir=None):
    nc = _get_nc()
    in_maps = make_in_maps(x, gamma)
    res = run_bass_kernel_spmd(
        nc, in_maps, list(range(NCORES)), trace=_trace, tmpdir=_tmpdir
    )
    outs = [np.asarray(res.results[i]["out"]) for i in range(NCORES)]
    full = np.concatenate(outs, axis=0).reshape(B, H, W, C)
    if _trace:
        return full, res
    return full
